# revision 2
# baseline (speedup 1.0000x reference)
"""Trainium2 Bass kernel for nn_Attention_9998683865539.

Multi-head attention (B=8, N=1024, C=768, H=12, HD=64, fp32), data-parallel
over the batch across 8 NeuronCores (one batch element per core, weights
replicated, no collectives).

Per-core dataflow (all matmul operands float32r = TF32-like, full PE rate):
  qkT  = (w_qkv_scaled.T).T @ xT        feature-major [1536, 1024]; the q-rows
                                        of w_qkv are pre-scaled by HD^-0.5 on
                                        the host so no separate scale op runs
  v    = x @ w_v.T                      token-major, packed per head with a
                                        trailing ones column (V' = [v | 1])
  per head h:
    S.T[k, q] = k_h @ q_h.T             K=HD matmuls straight from qkT slices
    P.T = exp(S.T)                      ScalarE, no max-subtraction (|S|<~7)
    [U.T; den] = V'.T @ P.T             M=65 matmul, PSUM accum over k-tiles;
                                        row 64 is the softmax denominator
    attnT_h = U.T * bcast(1/den)        DVE reciprocal + GPSIMD
                                        partition_broadcast + DVE multiply
  out = attnT.T @ w_proj.T + b_proj     bias folded in via a ones-row K=1
                                        matmul against a host-prepped b row

qk feature blocks 1..5, 7..11 are emitted as "filler" matmul chunks spliced
between head k-tile iterations so the PE stays busy while ACT drains exps.
"""
import sys

sys.path.insert(0, "/opt/trn_rl_repo")

import collections

import numpy as np

import concourse.bass as bass
import concourse.tile as tile
from concourse import bacc, mybir
from concourse import bass_utils

F32 = mybir.dt.float32
F32R = mybir.dt.float32r
EXP = mybir.ActivationFunctionType.Exp
MULT = mybir.AluOpType.mult

B = 8            # batch (one element per core)
C = 768          # channels
N = 1024         # tokens
H = 12           # heads
HD = 64          # head dim
SCALE = HD ** -0.5
NCT = C // 128   # 6 channel tiles
NTT = N // 128   # 8 token tiles
NQK = 12         # qk feature tiles (1536/128)
WV = H * (HD + 1)  # 780: per token-tile, 12 heads x (64 v + 1 ones)


def _build(reps=0, pt_bufs=4, wqs_bufs=4):
    nc = bacc.Bacc("TRN2", target_bir_lowering=False, debug=False)

    xT_d = nc.dram_tensor("xT", [C, N], F32, kind="ExternalInput").ap()
    wqb_d = nc.dram_tensor("wqb", [NQK, C, 128], F32, kind="ExternalInput").ap()
    wv_d = nc.dram_tensor("wv", [C, C], F32, kind="ExternalInput").ap()
    wp_d = nc.dram_tensor("wp", [C, C], F32, kind="ExternalInput").ap()
    bp_d = nc.dram_tensor("bp", [1, C], F32, kind="ExternalInput").ap()
    out_d = nc.dram_tensor("out", [N, C], F32, kind="ExternalOutput").ap()

    with tile.TileContext(nc) as tc:
        with (
            tc.tile_pool(name="big", bufs=1) as big,
            tc.tile_pool(name="ptp", bufs=pt_bufs) as ptp,
            tc.tile_pool(name="wkp", bufs=1) as wkp,
            tc.tile_pool(name="psp", bufs=2, space=bass.MemorySpace.PSUM) as psp,
        ):
            qk_t = big.tile([128, NQK * N], F32R)     # 48KB/part
            vp_t = big.tile([128, NTT * WV], F32R)    # 24.4KB/part
            attnT = big.tile([128, NCT * N], F32R)    # 24KB/part
            xr = big.tile([128, NCT * N], F32R)       # 24KB/part
            wv_t = big.tile([128, NCT * C], F32R)     # 18KB/part
            wp_t = big.tile([128, NCT * C], F32R)     # 18KB/part
            onesrow = wkp.tile([1, 128], F32R)
            ones12 = wkp.tile([128, H], F32)
            biasr = wkp.tile([1, C], F32R)
            tmp1 = wkp.tile([1, 128], F32, tag="recf", bufs=2)

            def emit():
                # constants
                nc.vector.memset(tmp1[:], 1.0)
                nc.vector.tensor_copy(onesrow[:], tmp1[:])
                nc.vector.memset(ones12[:], 1.0)

                def wq_load(ft):
                    wqs = wkp.tile(
                        [128, NCT * 128], F32R, tag="wqs", bufs=wqs_bufs
                    )
                    nc.gpsimd.dma_start(
                        wqs[:].rearrange("p (ct f) -> p ct f", f=128),
                        wqb_d[ft].rearrange("(ct p) f -> p ct f", p=128),
                    )
                    return wqs

                # DMA order: wq block 0 (per-ct chunks interleaved with xT
                # chunks) first, then block 6, then wv; wp + rest trail.
                w_first = wkp.tile([128, NCT * 128], F32R, tag="wqs", bufs=wqs_bufs, name="w_first")
                for ct in range(NCT):
                    nc.gpsimd.dma_start(
                        w_first[:, 128 * ct : 128 * (ct + 1)],
                        wqb_d[0][128 * ct : 128 * (ct + 1), :],
                    )
                    nc.gpsimd.dma_start(
                        xr[:, N * ct : N * (ct + 1)],
                        xT_d[128 * ct : 128 * (ct + 1), :],
                    )
                w_second = wq_load(6)
                nc.gpsimd.dma_start(
                    wv_t[:].rearrange("p (ct f) -> p ct f", f=C),
                    wv_d[:].rearrange("(ct p) f -> p ct f", p=128),
                )
                nc.gpsimd.dma_start(biasr[:], bp_d[:])

                def qk_compute(ft, wqs):
                    """qkT block ft, monolithic (pre-head phase)."""
                    ps = psp.tile([128, N], F32, tag="s")
                    for ct in range(NCT):
                        lhs = wqs[:, 128 * ct : 128 * (ct + 1)]
                        for qh in range(2):
                            nc.tensor.matmul(
                                ps[:, 512 * qh : 512 * (qh + 1)],
                                lhs,
                                xr[:, N * ct + 512 * qh : N * ct + 512 * (qh + 1)],
                                start=(ct == 0),
                                stop=(ct == NCT - 1),
                            )
                    nc.vector.tensor_copy(qk_t[:, N * ft : N * (ft + 1)], ps[:])

                filler = collections.deque()

                def queue_qk_chunks(ft, wqs):
                    """qkT block ft as 6 filler chunks (2 matmuls each),
                    accumulating in a u-tag PSUM slot."""
                    cell = {}

                    def chunk(ct):
                        if ct == 0:
                            cell["ps"] = psp.tile(
                                [128, N], F32, tag="u", name="qk_acc"
                            )
                        ps = cell["ps"]
                        lhs = wqs[:, 128 * ct : 128 * (ct + 1)]
                        for qh in range(2):
                            nc.tensor.matmul(
                                ps[:, 512 * qh : 512 * (qh + 1)],
                                lhs,
                                xr[:, N * ct + 512 * qh : N * ct + 512 * (qh + 1)],
                                start=(ct == 0),
                                stop=(ct == NCT - 1),
                            )
                        if ct == NCT - 1:
                            nc.vector.tensor_copy(
                                qk_t[:, N * ft : N * (ft + 1)], ps[:]
                            )

                    for ct in range(NCT):
                        filler.append(lambda ct=ct: chunk(ct))

                def v_block(m, tag="s"):
                    """v token-tile m -> vp [128, 780]: 12x(64 v cols + ones)."""
                    ps = psp.tile([128, N], F32, tag=tag, name="v_acc")
                    for ct in range(NCT):
                        lhs = xr[:, N * ct + 128 * m : N * ct + 128 * (m + 1)]
                        for nn, nw in ((0, 512), (512, 256)):
                            nc.tensor.matmul(
                                ps[:, nn : nn + nw],
                                lhs,
                                wv_t[:, C * ct + nn : C * ct + nn + nw],
                                start=(ct == 0),
                                stop=(ct == NCT - 1),
                            )
                    blk = vp_t[:, WV * m : WV * (m + 1)].rearrange(
                        "p (h c) -> p h c", c=HD + 1
                    )
                    nc.vector.tensor_copy(
                        blk[:, :, 0:HD],
                        ps[:, 0:C].rearrange("p (h c) -> p h c", c=HD),
                    )
                    nc.vector.tensor_copy(
                        blk[:, :, HD : HD + 1],
                        ones12[:].rearrange("p (h o) -> p h o", o=1),
                    )

                def head0_split():
                    """Head 0 in two waves of 4 k-tiles: scores+exp emitted
                    before that wave's v blocks, so ACT drains exps while the
                    PE computes v. Wave size matches pt_bufs."""
                    qft, po, kft = 0, 0, 6
                    wave = min(pt_bufs, 4)
                    ps_u = psp.tile([128, N], F32, tag="u")
                    for w0 in range(0, NTT, wave):
                        pts = []
                        for kt in range(w0, w0 + wave):
                            ps_s = psp.tile([128, N], F32, tag="s")
                            ksl = qk_t[
                                po : po + HD,
                                N * kft + 128 * kt : N * kft + 128 * (kt + 1),
                            ]
                            for qh in range(2):
                                nc.tensor.matmul(
                                    ps_s[:, 512 * qh : 512 * (qh + 1)],
                                    ksl,
                                    qk_t[
                                        po : po + HD,
                                        N * qft + 512 * qh : N * qft + 512 * (qh + 1),
                                    ],
                                    start=True,
                                    stop=True,
                                )
                            pt = ptp.tile([128, N], F32R, tag="pt")
                            nc.scalar.activation(pt[:], ps_s[:], EXP)
                            pts.append(pt)
                        for m in range(w0, w0 + wave):
                            v_block(m)
                        for kt in range(w0, w0 + wave):
                            vsl = vp_t[:, WV * kt : WV * kt + HD + 1]
                            for qh in range(2):
                                sl = slice(512 * qh, 512 * (qh + 1))
                                nc.tensor.matmul(
                                    ps_u[0:65, sl], vsl, pts[kt - w0][:, sl],
                                    start=(kt == 0), stop=(kt == NTT - 1),
                                )
                            if filler:
                                filler.popleft()()
                    uT = wkp.tile([128, N], F32, tag="uT", bufs=1)
                    nc.vector.tensor_copy(uT[0:65, :], ps_u[0:65, :])
                    rec_f = wkp.tile([1, N], F32, tag="recf2", bufs=1)
                    nc.vector.reciprocal(rec_f[:], uT[64:65, :])
                    bc = wkp.tile([64, N], F32, tag="bc", bufs=1)
                    nc.gpsimd.partition_broadcast(bc[:], rec_f[:])
                    nc.vector.tensor_tensor(
                        attnT[po : po + 64, N * qft : N * (qft + 1)],
                        uT[0:64, :],
                        bc[:],
                        op=MULT,
                    )

                def head(h):
                    qft, po = h // 2, 64 * (h % 2)
                    kft = 6 + h // 2
                    ps_u = psp.tile([128, N], F32, tag="u")
                    for kt in range(NTT):
                        ps_s = psp.tile([128, N], F32, tag="s")
                        ksl = qk_t[
                            po : po + HD,
                            N * kft + 128 * kt : N * kft + 128 * (kt + 1),
                        ]
                        for qh in range(2):
                            nc.tensor.matmul(
                                ps_s[:, 512 * qh : 512 * (qh + 1)],
                                ksl,
                                qk_t[
                                    po : po + HD,
                                    N * qft + 512 * qh : N * qft + 512 * (qh + 1),
                                ],
                                start=True,
                                stop=True,
                            )
                        pt = ptp.tile([128, N], F32R, tag="pt")
                        nc.scalar.activation(pt[:], ps_s[:], EXP)
                        vsl = vp_t[
                            :, WV * kt + (HD + 1) * h : WV * kt + (HD + 1) * (h + 1)
                        ]
                        for qh in range(2):
                            sl = slice(512 * qh, 512 * (qh + 1))
                            nc.tensor.matmul(
                                ps_u[0:65, sl], vsl, pt[:, sl],
                                start=(kt == 0), stop=(kt == NTT - 1),
                            )
                        if filler:
                            filler.popleft()()
                    # evacuate U+den, normalize off the PE:
                    # recip (DVE) -> partition_broadcast (gpsimd) -> mult (DVE)
                    uT = wkp.tile([128, N], F32, tag="uT", bufs=1)
                    nc.vector.tensor_copy(uT[0:65, :], ps_u[0:65, :])
                    rec_f = wkp.tile([1, N], F32, tag="recf2", bufs=1)
                    nc.vector.reciprocal(rec_f[:], uT[64:65, :])
                    bc = wkp.tile([64, N], F32, tag="bc", bufs=1)
                    nc.gpsimd.partition_broadcast(bc[:], rec_f[:])
                    nc.vector.tensor_tensor(
                        attnT[po : po + 64, N * qft : N * (qft + 1)],
                        uT[0:64, :],
                        bc[:],
                        op=MULT,
                    )

                # pre-head phase: blocks 0,6; head 0 split (v inside)
                qk_compute(0, w_first)
                qk_compute(6, w_second)

                # heads with deadline-scheduled qk fillers:
                # pair t (blocks t, 6+t) loads at head 2t-3, chunks during
                # heads 2t-2 / 2t-1, needed by head 2t.
                loads = {}
                loads[0] = (wq_load(1), wq_load(7))  # before head 0
                for h in range(H):
                    t = h // 2 + 1
                    if h % 2 == 0 and t <= 5:
                        wa, wb = loads.pop(h)
                        queue_qk_chunks(t, wa)
                        queue_qk_chunks(6 + t, wb)
                        if t + 1 <= 5:
                            loads[h + 2] = (wq_load(t + 1), wq_load(7 + t))
                    if h == 6:
                        nc.gpsimd.dma_start(
                            wp_t[:].rearrange("p (ct f) -> p ct f", f=C),
                            wp_d[:].rearrange("(ct p) f -> p ct f", p=128),
                        )
                    if h == 0:
                        head0_split()
                    else:
                        head(h)
                while filler:
                    filler.popleft()()

                # projection
                for m in range(NTT):
                    ps_o = psp.tile([128, N], F32, tag="s")
                    for ct in range(NCT - 1):
                        lhs = attnT[:, N * ct + 128 * m : N * ct + 128 * (m + 1)]
                        for nn, nw in ((0, 512), (512, 256)):
                            nc.tensor.matmul(
                                ps_o[:, nn : nn + nw],
                                lhs,
                                wp_t[:, C * ct + nn : C * ct + nn + nw],
                                start=(ct == 0),
                                stop=False,
                            )
                    for nn, nw in ((0, 512), (512, 256)):
                        nc.tensor.matmul(
                            ps_o[:, nn : nn + nw],
                            onesrow[0:1, :],
                            biasr[0:1, nn : nn + nw],
                            start=False,
                            stop=False,
                        )
                    ct = NCT - 1
                    lhs = attnT[:, N * ct + 128 * m : N * ct + 128 * (m + 1)]
                    for nn, nw in ((0, 512), (512, 256)):
                        nc.tensor.matmul(
                            ps_o[:, nn : nn + nw],
                            lhs,
                            wp_t[:, C * ct + nn : C * ct + nn + nw],
                            start=False,
                            stop=True,
                        )
                    o_sb = wkp.tile([128, C], F32, tag="osb", bufs=2)
                    nc.vector.tensor_copy(o_sb[:], ps_o[:, 0:C])
                    nc.sync.dma_start(out_d[128 * m : 128 * (m + 1), :], o_sb[:])

            if reps:
                with tc.For_i(0, reps, 1):
                    emit()
            else:
                emit()

    nc.compile()
    return nc


_CACHE = {}


def _get_nc():
    if "nc" not in _CACHE:
        _CACHE["nc"] = _build()
    return _CACHE["nc"]


def _host_prep(w_qkv, w_proj, b_proj):
    ws = np.asarray(w_qkv, dtype=np.float32).copy()
    ws[0:C] *= SCALE
    wt = np.ascontiguousarray(ws.T)  # [768, 2304]
    wqb = np.ascontiguousarray(
        wt[:, : 2 * C].reshape(C, NQK, 128).transpose(1, 0, 2)
    )
    wv = np.ascontiguousarray(wt[:, 2 * C :])
    wp = np.ascontiguousarray(np.asarray(w_proj, dtype=np.float32).T)
    bp = np.ascontiguousarray(np.asarray(b_proj, dtype=np.float32)[None, :])
    return wqb, wv, wp, bp


def kernel(x, w_qkv, w_proj, b_proj):
    x = np.asarray(x, dtype=np.float32)
    assert x.shape == (B, N, C), x.shape
    wqb, wv, wp, bp = _host_prep(w_qkv, w_proj, b_proj)
    in_maps = [
        {
            "xT": np.ascontiguousarray(x[b].T),
            "wqb": wqb,
            "wv": wv,
            "wp": wp,
            "bp": bp,
        }
        for b in range(B)
    ]
    nc = _get_nc()
    res = bass_utils.run_bass_kernel_spmd(nc, in_maps, core_ids=list(range(B)))
    return np.stack([np.asarray(res.results[b]["out"]) for b in range(B)]).astype(
        np.float32
    )


# revision 3
# speedup vs baseline: 1.0332x; 1.0332x over previous
"""Trainium2 Bass kernel for nn_Attention_9998683865539.

Multi-head attention (B=8, N=1024, C=768, H=12, HD=64, fp32), data-parallel
over the batch across 8 NeuronCores (one batch element per core, weights
replicated, no collectives).

Per-core dataflow (all matmul operands float32r = TF32-like, full PE rate):
  qkT  = (w_qkv_scaled.T).T @ xT        feature-major [1536, 1024]; the q-rows
                                        of w_qkv are pre-scaled by HD^-0.5 on
                                        the host so no separate scale op runs
  v    = x @ w_v.T                      token-major, packed per head with a
                                        trailing ones column (V' = [v | 1])
  per head h:
    S.T[k, q] = k_h @ q_h.T             K=HD matmuls straight from qkT slices
    P.T = exp(S.T)                      ScalarE, no max-subtraction (|S|<~7)
    [U.T; den] = V'.T @ P.T             M=65 matmul, PSUM accum over k-tiles;
                                        row 64 is the softmax denominator
    attnT_h = U.T * bcast(1/den)        DVE reciprocal + GPSIMD
                                        partition_broadcast + DVE multiply
  out = attnT.T @ w_proj.T + b_proj     bias folded in via a ones-row K=1
                                        matmul against a host-prepped b row

qk feature blocks 1..5, 7..11 are emitted as "filler" matmul chunks spliced
between head k-tile iterations so the PE stays busy while ACT drains exps.
"""
import sys

sys.path.insert(0, "/opt/trn_rl_repo")

import collections

import numpy as np

import concourse.bass as bass
import concourse.tile as tile
from concourse import bacc, mybir
from concourse import bass_utils

F32 = mybir.dt.float32
F32R = mybir.dt.float32r
EXP = mybir.ActivationFunctionType.Exp
MULT = mybir.AluOpType.mult

B = 8            # batch (one element per core)
C = 768          # channels
N = 1024         # tokens
H = 12           # heads
HD = 64          # head dim
SCALE = HD ** -0.5
NCT = C // 128   # 6 channel tiles
NTT = N // 128   # 8 token tiles
NQK = 12         # qk feature tiles (1536/128)
WV = H * (HD + 1)  # 780: per token-tile, 12 heads x (64 v + 1 ones)


def _build(reps=0, pt_bufs=4, wqs_bufs=4):
    nc = bacc.Bacc("TRN2", target_bir_lowering=False, debug=False)

    xT_d = nc.dram_tensor("xT", [C, N], F32, kind="ExternalInput").ap()
    wqb_d = nc.dram_tensor("wqb", [NQK, C, 128], F32, kind="ExternalInput").ap()
    wv_d = nc.dram_tensor("wv", [C, C], F32, kind="ExternalInput").ap()
    wp_d = nc.dram_tensor("wp", [C, C], F32, kind="ExternalInput").ap()
    bp_d = nc.dram_tensor("bp", [1, C], F32, kind="ExternalInput").ap()
    out_d = nc.dram_tensor("out", [N, C], F32, kind="ExternalOutput").ap()

    with tile.TileContext(nc) as tc:
        with (
            tc.tile_pool(name="big", bufs=1) as big,
            tc.tile_pool(name="ptp", bufs=pt_bufs) as ptp,
            tc.tile_pool(name="wkp", bufs=1) as wkp,
            tc.tile_pool(name="psp", bufs=2, space=bass.MemorySpace.PSUM) as psp,
        ):
            qk_t = big.tile([128, NQK * N], F32R)     # 48KB/part
            vp_t = big.tile([128, NTT * WV], F32R)    # 24.4KB/part
            attnT = big.tile([128, NCT * N], F32R)    # 24KB/part
            xr = big.tile([128, NCT * N], F32R)       # 24KB/part
            wv_t = big.tile([128, NCT * C], F32R)     # 18KB/part
            wp_t = big.tile([128, NCT * C], F32R)     # 18KB/part
            onesrow = wkp.tile([1, 128], F32R)
            ones12 = wkp.tile([128, H], F32)
            biasr = wkp.tile([1, C], F32R)
            tmp1 = wkp.tile([1, 128], F32, tag="recf", bufs=2)

            def emit():
                # constants
                nc.vector.memset(tmp1[:], 1.0)
                nc.vector.tensor_copy(onesrow[:], tmp1[:])
                nc.vector.memset(ones12[:], 1.0)

                def wq_load(ft):
                    wqs = wkp.tile(
                        [128, NCT * 128], F32R, tag="wqs", bufs=wqs_bufs
                    )
                    nc.gpsimd.dma_start(
                        wqs[:].rearrange("p (ct f) -> p ct f", f=128),
                        wqb_d[ft].rearrange("(ct p) f -> p ct f", p=128),
                    )
                    return wqs

                # DMA order: wq block 0 (per-ct chunks interleaved with xT
                # chunks) first, then block 6, then wv; wp + rest trail.
                w_first = wkp.tile([128, NCT * 128], F32R, tag="wqs", bufs=wqs_bufs, name="w_first")
                for ct in range(NCT):
                    nc.gpsimd.dma_start(
                        w_first[:, 128 * ct : 128 * (ct + 1)],
                        wqb_d[0][128 * ct : 128 * (ct + 1), :],
                    )
                    nc.gpsimd.dma_start(
                        xr[:, N * ct : N * (ct + 1)],
                        xT_d[128 * ct : 128 * (ct + 1), :],
                    )
                w_second = wq_load(6)
                nc.gpsimd.dma_start(
                    wv_t[:].rearrange("p (ct f) -> p ct f", f=C),
                    wv_d[:].rearrange("(ct p) f -> p ct f", p=128),
                )
                nc.gpsimd.dma_start(biasr[:], bp_d[:])

                def qk_compute(ft, wqs):
                    """qkT block ft, monolithic (pre-head phase)."""
                    ps = psp.tile([128, N], F32, tag="s")
                    for ct in range(NCT):
                        lhs = wqs[:, 128 * ct : 128 * (ct + 1)]
                        for qh in range(2):
                            nc.tensor.matmul(
                                ps[:, 512 * qh : 512 * (qh + 1)],
                                lhs,
                                xr[:, N * ct + 512 * qh : N * ct + 512 * (qh + 1)],
                                start=(ct == 0),
                                stop=(ct == NCT - 1),
                            )
                    nc.vector.tensor_copy(qk_t[:, N * ft : N * (ft + 1)], ps[:])

                filler = collections.deque()

                def queue_qk_chunks(ft, wqs):
                    """qkT block ft as 6 filler chunks (2 matmuls each),
                    accumulating in a u-tag PSUM slot."""
                    cell = {}

                    def chunk(ct):
                        if ct == 0:
                            cell["ps"] = psp.tile(
                                [128, N], F32, tag="u", name="qk_acc"
                            )
                        ps = cell["ps"]
                        lhs = wqs[:, 128 * ct : 128 * (ct + 1)]
                        for qh in range(2):
                            nc.tensor.matmul(
                                ps[:, 512 * qh : 512 * (qh + 1)],
                                lhs,
                                xr[:, N * ct + 512 * qh : N * ct + 512 * (qh + 1)],
                                start=(ct == 0),
                                stop=(ct == NCT - 1),
                            )
                        if ct == NCT - 1:
                            nc.vector.tensor_copy(
                                qk_t[:, N * ft : N * (ft + 1)], ps[:]
                            )

                    for ct in range(NCT):
                        filler.append(lambda ct=ct: chunk(ct))

                def v_block(m, tag="s"):
                    """v token-tile m -> vp [128, 780]: 12x(64 v cols + ones)."""
                    ps = psp.tile([128, N], F32, tag=tag, name="v_acc")
                    for ct in range(NCT):
                        lhs = xr[:, N * ct + 128 * m : N * ct + 128 * (m + 1)]
                        for nn, nw in ((0, 512), (512, 256)):
                            nc.tensor.matmul(
                                ps[:, nn : nn + nw],
                                lhs,
                                wv_t[:, C * ct + nn : C * ct + nn + nw],
                                start=(ct == 0),
                                stop=(ct == NCT - 1),
                            )
                    blk = vp_t[:, WV * m : WV * (m + 1)].rearrange(
                        "p (h c) -> p h c", c=HD + 1
                    )
                    nc.vector.tensor_copy(
                        blk[:, :, 0:HD],
                        ps[:, 0:C].rearrange("p (h c) -> p h c", c=HD),
                    )
                    nc.vector.tensor_copy(
                        blk[:, :, HD : HD + 1],
                        ones12[:].rearrange("p (h o) -> p h o", o=1),
                    )

                def head0_split():
                    """Head 0 in two waves of 4 k-tiles: scores+exp emitted
                    before that wave's v blocks, so ACT drains exps while the
                    PE computes v. Wave size matches pt_bufs."""
                    qft, po, kft = 0, 0, 6
                    wave = min(pt_bufs, 4)
                    ps_u = psp.tile([128, N], F32, tag="u")
                    for w0 in range(0, NTT, wave):
                        pts = []
                        for kt in range(w0, w0 + wave):
                            ps_s = psp.tile([128, N], F32, tag="s")
                            ksl = qk_t[
                                po : po + HD,
                                N * kft + 128 * kt : N * kft + 128 * (kt + 1),
                            ]
                            for qh in range(2):
                                nc.tensor.matmul(
                                    ps_s[:, 512 * qh : 512 * (qh + 1)],
                                    ksl,
                                    qk_t[
                                        po : po + HD,
                                        N * qft + 512 * qh : N * qft + 512 * (qh + 1),
                                    ],
                                    start=True,
                                    stop=True,
                                )
                            pt = ptp.tile([128, N], F32R, tag="pt")
                            nc.scalar.activation(pt[:], ps_s[:], EXP)
                            pts.append(pt)
                        for m in range(w0, w0 + wave):
                            v_block(m)
                        for kt in range(w0, w0 + wave):
                            vsl = vp_t[:, WV * kt : WV * kt + HD + 1]
                            for qh in range(2):
                                sl = slice(512 * qh, 512 * (qh + 1))
                                nc.tensor.matmul(
                                    ps_u[0:65, sl], vsl, pts[kt - w0][:, sl],
                                    start=(kt == 0), stop=(kt == NTT - 1),
                                )
                            if filler:
                                filler.popleft()()
                    uT = wkp.tile([128, N], F32, tag="uT", bufs=1)
                    nc.vector.tensor_copy(uT[0:65, :], ps_u[0:65, :])
                    rec_f = wkp.tile([1, N], F32, tag="recf2", bufs=1)
                    nc.vector.reciprocal(rec_f[:], uT[64:65, :])
                    bc = wkp.tile([64, N], F32, tag="bc", bufs=1)
                    nc.gpsimd.partition_broadcast(bc[:], rec_f[:])
                    nc.vector.tensor_tensor(
                        attnT[po : po + 64, N * qft : N * (qft + 1)],
                        uT[0:64, :],
                        bc[:],
                        op=MULT,
                    )

                def head(h):
                    qft, po = h // 2, 64 * (h % 2)
                    kft = 6 + h // 2
                    ps_u = psp.tile([128, N], F32, tag="u")
                    for kt in range(NTT):
                        ps_s = psp.tile([128, N], F32, tag="s")
                        ksl = qk_t[
                            po : po + HD,
                            N * kft + 128 * kt : N * kft + 128 * (kt + 1),
                        ]
                        for qh in range(2):
                            nc.tensor.matmul(
                                ps_s[:, 512 * qh : 512 * (qh + 1)],
                                ksl,
                                qk_t[
                                    po : po + HD,
                                    N * qft + 512 * qh : N * qft + 512 * (qh + 1),
                                ],
                                start=True,
                                stop=True,
                            )
                        pt = ptp.tile([128, N], F32R, tag="pt")
                        nc.scalar.activation(pt[:], ps_s[:], EXP)
                        vsl = vp_t[
                            :, WV * kt + (HD + 1) * h : WV * kt + (HD + 1) * (h + 1)
                        ]
                        for qh in range(2):
                            sl = slice(512 * qh, 512 * (qh + 1))
                            nc.tensor.matmul(
                                ps_u[0:65, sl], vsl, pt[:, sl],
                                start=(kt == 0), stop=(kt == NTT - 1),
                            )
                        if filler and (kt >= 2 or h % 2 == 1):
                            filler.popleft()()
                    # evacuate U+den, normalize off the PE:
                    # recip (DVE) -> partition_broadcast (gpsimd) -> mult (DVE)
                    uT = wkp.tile([128, N], F32, tag="uT", bufs=1)
                    nc.vector.tensor_copy(uT[0:65, :], ps_u[0:65, :])
                    rec_f = wkp.tile([1, N], F32, tag="recf2", bufs=1)
                    nc.vector.reciprocal(rec_f[:], uT[64:65, :])
                    bc = wkp.tile([64, N], F32, tag="bc", bufs=1)
                    nc.gpsimd.partition_broadcast(bc[:], rec_f[:])
                    nc.vector.tensor_tensor(
                        attnT[po : po + 64, N * qft : N * (qft + 1)],
                        uT[0:64, :],
                        bc[:],
                        op=MULT,
                    )

                # pre-head phase: blocks 0,6; head 0 split (v inside)
                qk_compute(0, w_first)
                qk_compute(6, w_second)

                # heads with deadline-scheduled qk fillers:
                # pair t (blocks t, 6+t) loads at head 2t-3, chunks during
                # heads 2t-2 / 2t-1, needed by head 2t.
                loads = {}
                loads[0] = (wq_load(1), wq_load(7))  # before head 0
                for h in range(H):
                    t = h // 2 + 1
                    if h % 2 == 0 and t <= 5:
                        wa, wb = loads.pop(h)
                        queue_qk_chunks(t, wa)
                        queue_qk_chunks(6 + t, wb)
                        if t + 1 <= 5:
                            loads[h + 2] = (wq_load(t + 1), wq_load(7 + t))
                    if h == 6:
                        nc.gpsimd.dma_start(
                            wp_t[:].rearrange("p (ct f) -> p ct f", f=C),
                            wp_d[:].rearrange("(ct p) f -> p ct f", p=128),
                        )
                    if h == 0:
                        head0_split()
                    else:
                        head(h)
                while filler:
                    filler.popleft()()

                # projection
                for m in range(NTT):
                    ps_o = psp.tile([128, N], F32, tag="s")
                    for ct in range(NCT - 1):
                        lhs = attnT[:, N * ct + 128 * m : N * ct + 128 * (m + 1)]
                        for nn, nw in ((0, 512), (512, 256)):
                            nc.tensor.matmul(
                                ps_o[:, nn : nn + nw],
                                lhs,
                                wp_t[:, C * ct + nn : C * ct + nn + nw],
                                start=(ct == 0),
                                stop=False,
                            )
                    for nn, nw in ((0, 512), (512, 256)):
                        nc.tensor.matmul(
                            ps_o[:, nn : nn + nw],
                            onesrow[0:1, :],
                            biasr[0:1, nn : nn + nw],
                            start=False,
                            stop=False,
                        )
                    ct = NCT - 1
                    lhs = attnT[:, N * ct + 128 * m : N * ct + 128 * (m + 1)]
                    for nn, nw in ((0, 512), (512, 256)):
                        nc.tensor.matmul(
                            ps_o[:, nn : nn + nw],
                            lhs,
                            wp_t[:, C * ct + nn : C * ct + nn + nw],
                            start=False,
                            stop=True,
                        )
                    o_sb = wkp.tile([128, C], F32, tag="osb", bufs=2)
                    nc.vector.tensor_copy(o_sb[:], ps_o[:, 0:C])
                    nc.sync.dma_start(out_d[128 * m : 128 * (m + 1), :], o_sb[:])

            if reps:
                with tc.For_i(0, reps, 1):
                    emit()
            else:
                emit()

    nc.compile()
    return nc


_CACHE = {}


def _get_nc():
    if "nc" not in _CACHE:
        _CACHE["nc"] = _build()
    return _CACHE["nc"]


def _host_prep(w_qkv, w_proj, b_proj):
    ws = np.asarray(w_qkv, dtype=np.float32).copy()
    ws[0:C] *= SCALE
    wt = np.ascontiguousarray(ws.T)  # [768, 2304]
    wqb = np.ascontiguousarray(
        wt[:, : 2 * C].reshape(C, NQK, 128).transpose(1, 0, 2)
    )
    wv = np.ascontiguousarray(wt[:, 2 * C :])
    wp = np.ascontiguousarray(np.asarray(w_proj, dtype=np.float32).T)
    bp = np.ascontiguousarray(np.asarray(b_proj, dtype=np.float32)[None, :])
    return wqb, wv, wp, bp


def kernel(x, w_qkv, w_proj, b_proj):
    x = np.asarray(x, dtype=np.float32)
    assert x.shape == (B, N, C), x.shape
    wqb, wv, wp, bp = _host_prep(w_qkv, w_proj, b_proj)
    in_maps = [
        {
            "xT": np.ascontiguousarray(x[b].T),
            "wqb": wqb,
            "wv": wv,
            "wp": wp,
            "bp": bp,
        }
        for b in range(B)
    ]
    nc = _get_nc()
    res = bass_utils.run_bass_kernel_spmd(nc, in_maps, core_ids=list(range(B)))
    return np.stack([np.asarray(res.results[b]["out"]) for b in range(B)]).astype(
        np.float32
    )


# revision 4
# speedup vs baseline: 1.0505x; 1.0167x over previous
"""Trainium2 Bass kernel for nn_Attention_9998683865539.

Multi-head attention (B=8, N=1024, C=768, H=12, HD=64, fp32), data-parallel
over the batch across 8 NeuronCores (one batch element per core, weights
replicated, no collectives).

Per-core dataflow (all matmul operands float32r = TF32-like, full PE rate):
  qkT  = (w_qkv_scaled.T).T @ xT        feature-major [1536, 1024]; the q-rows
                                        of w_qkv are pre-scaled by HD^-0.5 on
                                        the host so no separate scale op runs
  v    = x @ w_v.T                      token-major, packed per head with a
                                        trailing ones column (V' = [v | 1])
  per head h:
    S.T[k, q] = k_h @ q_h.T             K=HD matmuls straight from qkT slices
    P.T = exp(S.T)                      ScalarE, no max-subtraction (|S|<~7)
    [U.T; den] = V'.T @ P.T             M=65 matmul, PSUM accum over k-tiles;
                                        row 64 is the softmax denominator
    attnT_h = U.T * bcast(1/den)        DVE reciprocal + GPSIMD
                                        partition_broadcast + DVE multiply
  out = attnT.T @ w_proj.T + b_proj     bias folded in via a ones-row K=1
                                        matmul against a host-prepped b row

qk feature blocks 1..5, 7..11 are emitted as "filler" matmul chunks spliced
between head k-tile iterations so the PE stays busy while ACT drains exps.
"""
import sys

sys.path.insert(0, "/opt/trn_rl_repo")

import collections

import numpy as np

import concourse.bass as bass
import concourse.tile as tile
from concourse import bacc, mybir
from concourse import bass_utils

F32 = mybir.dt.float32
F32R = mybir.dt.float32r
EXP = mybir.ActivationFunctionType.Exp
MULT = mybir.AluOpType.mult

B = 8            # batch (one element per core)
C = 768          # channels
N = 1024         # tokens
H = 12           # heads
HD = 64          # head dim
SCALE = HD ** -0.5
NCT = C // 128   # 6 channel tiles
NTT = N // 128   # 8 token tiles
NQK = 12         # qk feature tiles (1536/128)
WV = H * (HD + 1)  # 780: per token-tile, 12 heads x (64 v + 1 ones)


def _build(reps=0, pt_bufs=4, wqs_bufs=4):
    nc = bacc.Bacc("TRN2", target_bir_lowering=False, debug=False)

    xT_d = nc.dram_tensor("xT", [C, N], F32, kind="ExternalInput").ap()
    wqb_d = nc.dram_tensor("wqb", [NQK, C, 128], F32, kind="ExternalInput").ap()
    wv_d = nc.dram_tensor("wv", [C, C], F32, kind="ExternalInput").ap()
    wp_d = nc.dram_tensor("wp", [C, C], F32, kind="ExternalInput").ap()
    bp_d = nc.dram_tensor("bp", [128, C], F32, kind="ExternalInput").ap()
    out_d = nc.dram_tensor("out", [N, C], F32, kind="ExternalOutput").ap()

    with tile.TileContext(nc) as tc:
        with (
            tc.tile_pool(name="big", bufs=1) as big,
            tc.tile_pool(name="ptp", bufs=pt_bufs) as ptp,
            tc.tile_pool(name="wkp", bufs=1) as wkp,
            tc.tile_pool(name="psp", bufs=2, space=bass.MemorySpace.PSUM) as psp,
        ):
            qk_t = big.tile([128, NQK * N], F32R)     # 48KB/part
            vp_t = big.tile([128, NTT * WV], F32R)    # 24.4KB/part
            attnT = big.tile([128, NCT * N], F32R)    # 24KB/part
            xr = big.tile([128, NCT * N], F32R)       # 24KB/part
            wv_t = big.tile([128, NCT * C], F32R)     # 18KB/part
            wp_t = big.tile([128, NCT * C], F32R)     # 18KB/part
            ones12 = wkp.tile([128, H], F32)
            bias_sb = wkp.tile([128, C], F32)

            def emit():
                # constants
                nc.vector.memset(ones12[:], 1.0)

                def wq_load(ft):
                    wqs = wkp.tile(
                        [128, NCT * 128], F32R, tag="wqs", bufs=wqs_bufs
                    )
                    nc.gpsimd.dma_start(
                        wqs[:].rearrange("p (ct f) -> p ct f", f=128),
                        wqb_d[ft].rearrange("(ct p) f -> p ct f", p=128),
                    )
                    return wqs

                # DMA order: wq block 0 (per-ct chunks interleaved with xT
                # chunks) first, then block 6, then wv; wp + rest trail.
                w_first = wkp.tile([128, NCT * 128], F32R, tag="wqs", bufs=wqs_bufs, name="w_first")
                for ct in range(NCT):
                    nc.gpsimd.dma_start(
                        w_first[:, 128 * ct : 128 * (ct + 1)],
                        wqb_d[0][128 * ct : 128 * (ct + 1), :],
                    )
                    nc.gpsimd.dma_start(
                        xr[:, N * ct : N * (ct + 1)],
                        xT_d[128 * ct : 128 * (ct + 1), :],
                    )
                w_second = wq_load(6)
                nc.gpsimd.dma_start(
                    wv_t[:].rearrange("p (ct f) -> p ct f", f=C),
                    wv_d[:].rearrange("(ct p) f -> p ct f", p=128),
                )
                nc.sync.dma_start(bias_sb[:], bp_d[:])

                def qk_compute(ft, wqs):
                    """qkT block ft, monolithic (pre-head phase)."""
                    ps = psp.tile([128, N], F32, tag="s")
                    for ct in range(NCT):
                        lhs = wqs[:, 128 * ct : 128 * (ct + 1)]
                        for qh in range(2):
                            nc.tensor.matmul(
                                ps[:, 512 * qh : 512 * (qh + 1)],
                                lhs,
                                xr[:, N * ct + 512 * qh : N * ct + 512 * (qh + 1)],
                                start=(ct == 0),
                                stop=(ct == NCT - 1),
                            )
                    nc.vector.tensor_copy(qk_t[:, N * ft : N * (ft + 1)], ps[:])

                filler = collections.deque()

                def queue_qk_chunks(ft, wqs):
                    """qkT block ft as 6 filler chunks (2 matmuls each),
                    accumulating in a u-tag PSUM slot."""
                    cell = {}

                    def chunk(ct):
                        if ct == 0:
                            cell["ps"] = psp.tile(
                                [128, N], F32, tag="u", name="qk_acc"
                            )
                        ps = cell["ps"]
                        lhs = wqs[:, 128 * ct : 128 * (ct + 1)]
                        for qh in range(2):
                            nc.tensor.matmul(
                                ps[:, 512 * qh : 512 * (qh + 1)],
                                lhs,
                                xr[:, N * ct + 512 * qh : N * ct + 512 * (qh + 1)],
                                start=(ct == 0),
                                stop=(ct == NCT - 1),
                            )
                        if ct == NCT - 1:
                            nc.vector.tensor_copy(
                                qk_t[:, N * ft : N * (ft + 1)], ps[:]
                            )

                    for ct in range(NCT):
                        filler.append(lambda ct=ct: chunk(ct))

                def v_block(m, tag="s"):
                    """v token-tile m -> vp [128, 780]: 12x(64 v cols + ones)."""
                    ps = psp.tile([128, N], F32, tag=tag, name="v_acc")
                    for ct in range(NCT):
                        lhs = xr[:, N * ct + 128 * m : N * ct + 128 * (m + 1)]
                        for nn, nw in ((0, 512), (512, 256)):
                            nc.tensor.matmul(
                                ps[:, nn : nn + nw],
                                lhs,
                                wv_t[:, C * ct + nn : C * ct + nn + nw],
                                start=(ct == 0),
                                stop=(ct == NCT - 1),
                            )
                    blk = vp_t[:, WV * m : WV * (m + 1)].rearrange(
                        "p (h c) -> p h c", c=HD + 1
                    )
                    nc.vector.tensor_copy(
                        blk[:, :, 0:HD],
                        ps[:, 0:C].rearrange("p (h c) -> p h c", c=HD),
                    )
                    nc.vector.tensor_copy(
                        blk[:, :, HD : HD + 1],
                        ones12[:].rearrange("p (h o) -> p h o", o=1),
                    )

                def head0_split():
                    """Head 0 in two waves of 4 k-tiles: scores+exp emitted
                    before that wave's v blocks, so ACT drains exps while the
                    PE computes v. Wave size matches pt_bufs."""
                    qft, po, kft = 0, 0, 6
                    wave = min(pt_bufs, 4)
                    ps_u = psp.tile([128, N], F32, tag="u")
                    for w0 in range(0, NTT, wave):
                        pts = []
                        for kt in range(w0, w0 + wave):
                            ps_s = psp.tile([128, N], F32, tag="s")
                            ksl = qk_t[
                                po : po + HD,
                                N * kft + 128 * kt : N * kft + 128 * (kt + 1),
                            ]
                            for qh in range(2):
                                nc.tensor.matmul(
                                    ps_s[:, 512 * qh : 512 * (qh + 1)],
                                    ksl,
                                    qk_t[
                                        po : po + HD,
                                        N * qft + 512 * qh : N * qft + 512 * (qh + 1),
                                    ],
                                    start=True,
                                    stop=True,
                                )
                            pt = ptp.tile([128, N], F32R, tag="pt")
                            nc.scalar.activation(pt[:], ps_s[:], EXP)
                            pts.append(pt)
                        for m in range(w0, w0 + wave):
                            v_block(m)
                        for kt in range(w0, w0 + wave):
                            vsl = vp_t[:, WV * kt : WV * kt + HD + 1]
                            for qh in range(2):
                                sl = slice(512 * qh, 512 * (qh + 1))
                                nc.tensor.matmul(
                                    ps_u[0:65, sl], vsl, pts[kt - w0][:, sl],
                                    start=(kt == 0), stop=(kt == NTT - 1),
                                )
                            if filler:
                                filler.popleft()()
                    uT = wkp.tile([128, N], F32, tag="uT", bufs=1)
                    nc.vector.tensor_copy(uT[0:65, :], ps_u[0:65, :])
                    rec_f = wkp.tile([1, N], F32, tag="recf2", bufs=1)
                    nc.vector.reciprocal(rec_f[:], uT[64:65, :])
                    bc = wkp.tile([64, N], F32, tag="bc", bufs=1)
                    nc.gpsimd.partition_broadcast(bc[:], rec_f[:])
                    nc.vector.tensor_tensor(
                        attnT[po : po + 64, N * qft : N * (qft + 1)],
                        uT[0:64, :],
                        bc[:],
                        op=MULT,
                    )

                def head(h):
                    qft, po = h // 2, 64 * (h % 2)
                    kft = 6 + h // 2
                    ps_u = psp.tile([128, N], F32, tag="u")
                    for kt in range(NTT):
                        ps_s = psp.tile([128, N], F32, tag="s")
                        ksl = qk_t[
                            po : po + HD,
                            N * kft + 128 * kt : N * kft + 128 * (kt + 1),
                        ]
                        for qh in range(2):
                            nc.tensor.matmul(
                                ps_s[:, 512 * qh : 512 * (qh + 1)],
                                ksl,
                                qk_t[
                                    po : po + HD,
                                    N * qft + 512 * qh : N * qft + 512 * (qh + 1),
                                ],
                                start=True,
                                stop=True,
                            )
                        pt = ptp.tile([128, N], F32R, tag="pt")
                        nc.scalar.activation(pt[:], ps_s[:], EXP)
                        vsl = vp_t[
                            :, WV * kt + (HD + 1) * h : WV * kt + (HD + 1) * (h + 1)
                        ]
                        for qh in range(2):
                            sl = slice(512 * qh, 512 * (qh + 1))
                            nc.tensor.matmul(
                                ps_u[0:65, sl], vsl, pt[:, sl],
                                start=(kt == 0), stop=(kt == NTT - 1),
                            )
                        if filler and (kt >= 2 or h % 2 == 1):
                            filler.popleft()()
                    # evacuate U+den, normalize off the PE:
                    # recip (DVE) -> partition_broadcast (gpsimd) -> mult (DVE)
                    uT = wkp.tile([128, N], F32, tag="uT", bufs=1)
                    nc.vector.tensor_copy(uT[0:65, :], ps_u[0:65, :])
                    rec_f = wkp.tile([1, N], F32, tag="recf2", bufs=1)
                    nc.vector.reciprocal(rec_f[:], uT[64:65, :])
                    bc = wkp.tile([64, N], F32, tag="bc", bufs=1)
                    nc.gpsimd.partition_broadcast(bc[:], rec_f[:])
                    nc.vector.tensor_tensor(
                        attnT[po : po + 64, N * qft : N * (qft + 1)],
                        uT[0:64, :],
                        bc[:],
                        op=MULT,
                    )

                # pre-head phase: blocks 0,6; head 0 split (v inside)
                qk_compute(0, w_first)
                qk_compute(6, w_second)

                # heads with deadline-scheduled qk fillers:
                # pair t (blocks t, 6+t) loads at head 2t-3, chunks during
                # heads 2t-2 / 2t-1, needed by head 2t.
                loads = {}
                loads[0] = (wq_load(1), wq_load(7))  # before head 0
                for h in range(H):
                    t = h // 2 + 1
                    if h % 2 == 0 and t <= 5:
                        wa, wb = loads.pop(h)
                        queue_qk_chunks(t, wa)
                        queue_qk_chunks(6 + t, wb)
                        if t + 1 <= 5:
                            loads[h + 2] = (wq_load(t + 1), wq_load(7 + t))
                    if h == 6:
                        nc.gpsimd.dma_start(
                            wp_t[:].rearrange("p (ct f) -> p ct f", f=C),
                            wp_d[:].rearrange("(ct p) f -> p ct f", p=128),
                        )
                    if h == 0:
                        head0_split()
                    else:
                        head(h)
                while filler:
                    filler.popleft()()

                # projection
                for m in range(NTT):
                    ps_o = psp.tile([128, N], F32, tag="s")
                    for ct in range(NCT - 1):
                        lhs = attnT[:, N * ct + 128 * m : N * ct + 128 * (m + 1)]
                        for nn, nw in ((0, 512), (512, 256)):
                            nc.tensor.matmul(
                                ps_o[:, nn : nn + nw],
                                lhs,
                                wp_t[:, C * ct + nn : C * ct + nn + nw],
                                start=(ct == 0),
                                stop=False,
                            )
                    ct = NCT - 1
                    lhs = attnT[:, N * ct + 128 * m : N * ct + 128 * (m + 1)]
                    for nn, nw in ((0, 512), (512, 256)):
                        nc.tensor.matmul(
                            ps_o[:, nn : nn + nw],
                            lhs,
                            wp_t[:, C * ct + nn : C * ct + nn + nw],
                            start=False,
                            stop=True,
                        )
                    o_sb = wkp.tile([128, C], F32, tag="osb", bufs=2)
                    nc.vector.tensor_tensor(
                        o_sb[:], ps_o[:, 0:C], bias_sb[:], op=mybir.AluOpType.add
                    )
                    nc.sync.dma_start(out_d[128 * m : 128 * (m + 1), :], o_sb[:])

            if reps:
                with tc.For_i(0, reps, 1):
                    emit()
            else:
                emit()

    nc.compile()
    return nc


_CACHE = {}


def _get_nc():
    if "nc" not in _CACHE:
        _CACHE["nc"] = _build()
    return _CACHE["nc"]


def _host_prep(w_qkv, w_proj, b_proj):
    ws = np.asarray(w_qkv, dtype=np.float32).copy()
    ws[0:C] *= SCALE
    wt = np.ascontiguousarray(ws.T)  # [768, 2304]
    wqb = np.ascontiguousarray(
        wt[:, : 2 * C].reshape(C, NQK, 128).transpose(1, 0, 2)
    )
    wv = np.ascontiguousarray(wt[:, 2 * C :])
    wp = np.ascontiguousarray(np.asarray(w_proj, dtype=np.float32).T)
    bp = np.ascontiguousarray(np.tile(np.asarray(b_proj, dtype=np.float32)[None, :], (128, 1)))
    return wqb, wv, wp, bp


def kernel(x, w_qkv, w_proj, b_proj):
    x = np.asarray(x, dtype=np.float32)
    assert x.shape == (B, N, C), x.shape
    wqb, wv, wp, bp = _host_prep(w_qkv, w_proj, b_proj)
    in_maps = [
        {
            "xT": np.ascontiguousarray(x[b].T),
            "wqb": wqb,
            "wv": wv,
            "wp": wp,
            "bp": bp,
        }
        for b in range(B)
    ]
    nc = _get_nc()
    res = bass_utils.run_bass_kernel_spmd(nc, in_maps, core_ids=list(range(B)))
    return np.stack([np.asarray(res.results[b]["out"]) for b in range(B)]).astype(
        np.float32
    )
